# revision 47
# baseline (speedup 1.0000x reference)
"""Causal self-attention (QK-RMSNorm + RoPE) on 8 Trainium2 NeuronCores.

Problem: x[2,2048,2048], Wq/Wk/Wv/Wo [2048,2048], 16 heads, head_dim 128.

Sharding: core c handles batch b=c//4 and head group g=c%4 (4 heads,
model cols [512g:512g+512)).  Q/K/V projections are computed in one pass
from host-pre-transposed xT (contraction dim on partitions); q/k get
RMS-norm + RoPE row-wise, are transposed per head on the PE, and stay
resident in SBUF (no DRAM round trip).  Attention uses transposed scores
(eT = exp(scale * kT_blk.T @ qT_chunk)), so the AV matmul (lhsT=v,
rhs=eT) directly yields the transposed attention output yT[d, i] that
o_proj consumes.  Scores for each key block only stream the causally
valid query window (rounded to 128), with a single 128x128 triangular
mask tile for the diagonal sub-blocks.  The softmax denominator comes
from a ones-lhsT matmul over eT, its reciprocal is broadcast across
partitions with a K=1 PE matmul.  Per-batch groups of 4 cores AllGather
their yT head shards per i-chunk; o_proj for chunk ic-1 is emitted
right after the AllGather of chunk ic so it overlaps attention.  Each
core computes a 512-column slice of y = attn @ Wo.T in transposed
layout; the host de-transposes and concatenates.
"""

import math
from contextlib import ExitStack

import numpy as np

import concourse.bass as bass
import concourse.bacc as bacc
import concourse.tile as tile
from concourse import mybir
from concourse.bass_utils import run_bass_kernel_spmd
from concourse.masks import make_identity

P = 128
D = 2048
S = 2048
HD = 128              # head dim
NHL = 4               # heads per core
GW = NHL * HD         # 512, per-core width of head group
CT = D // P           # 16 contraction tiles
ICH = 4               # i-chunks of 512 positions
NCORES = 8
F32 = mybir.dt.float32
F16 = mybir.dt.float16
F32R = mybir.dt.float32r
SCALE = 1.0 / math.sqrt(HD)
EPS = 1.1920928955078125e-07
# chunk-3 o_proj consumption order: the early-gathered head pairs first
MT3_ORDER = [mt for mt in range(CT) if mt % 4 < 2] + [mt for mt in range(CT) if mt % 4 >= 2]

_program_cache = {}


def build_program():
    if "nc" in _program_cache:
        return _program_cache["nc"]

    nc = bacc.Bacc("TRN2", target_bir_lowering=False, debug=False, num_devices=NCORES)

    xt_in = nc.dram_tensor("xt", [D, S], F16, kind="ExternalInput")
    wq_in = nc.dram_tensor("wq", [D, GW], F16, kind="ExternalInput")
    wk_in = nc.dram_tensor("wk", [D, GW], F16, kind="ExternalInput")
    wv_in = nc.dram_tensor("wv", [D, GW], F16, kind="ExternalInput")
    wo_in = nc.dram_tensor("wo", [D, GW], F16, kind="ExternalInput")
    cos_in = nc.dram_tensor("cos", [S, HD // 2], F16, kind="ExternalInput")
    sin_in = nc.dram_tensor("sin", [S, HD // 2], F16, kind="ExternalInput")
    yt_out = nc.dram_tensor("yt_out", [GW, S], F16, kind="ExternalOutput")

    with tile.TileContext(nc) as tc:
        with ExitStack() as ctx:
            const = ctx.enter_context(tc.tile_pool(name="const", bufs=1))
            dram = ctx.enter_context(tc.tile_pool(name="dram", bufs=1, space="DRAM"))

            ident = const.tile([P, P], F16, name="ident")
            make_identity(nc, ident)
            eps_t = const.tile([P, 1], F32, name="eps_t")
            nc.vector.memset(eps_t[:], EPS)
            neg1_t = const.tile([P, 1], F32, name="neg1_t")
            nc.vector.memset(neg1_t[:], -1.0)
            ones_f = const.tile([P, P], F32, name="ones_f")
            nc.vector.memset(ones_f[:], 1.0)
            # full-width ones: den matmul replicates the column sums across
            # all 128 output partitions (same stream cost as 2 partitions)
            ones16 = const.tile([P, P], F16, name="ones16")
            nc.scalar.copy(ones16[:], ones_f[:])
            # tri_m[j, i] = 1 where i >= j (valid), else 0 — diagonal block mask
            tri_m = const.tile([P, P], F16, name="tri_m")
            nc.gpsimd.memset(tri_m[:], 1.0)
            nc.gpsimd.affine_select(
                out=tri_m[:], in_=tri_m[:],
                compare_op=mybir.AluOpType.is_ge,
                fill=0.0,
                base=0,
                pattern=[[1, P]],
                channel_multiplier=-1,
            )

            cos_sb = const.tile([P, CT, HD // 2], F16, name="cos_sb")
            sin_sb = const.tile([P, CT, HD // 2], F16, name="sin_sb")

            yt_ics = [dram.tile([GW, 512], F16, name=f"yt_ic{i}") for i in range(ICH)]
            ag_ics = [
                dram.tile([4 * GW, 512], F16, name=f"ag_ic{i}")
                for i in range(ICH)
            ]


            # persistent SBUF across phases: transposed q/k, v, and Wo
            kv_pool = ctx.enter_context(tc.tile_pool(name="kv_pool", bufs=1))
            qt_sb = kv_pool.tile([P, NHL, S], F16, name="qt_sb")
            kt_sb = kv_pool.tile([P, NHL, S], F16, name="kt_sb")
            v_sb = kv_pool.tile([P, CT, GW], F16, name="v_sb")
            wo_sb = kv_pool.tile([P, CT, GW], F16, name="wo_sb")

            def norm_rope(rope, ps, ibg, t):
                """RMS-norm stats + RoPE on a projection PSUM tile; returns
                the rotated+normalized [P, GW] f16 tile."""
                qs = rope.tile([P, GW], F16, name=f"{t}s{ibg}", tag=f"{t}s")
                nc.scalar.copy(qs[:], ps[:])
                sq = rope.tile([P, GW], F16, name=f"{t}sq{ibg}", tag=f"{t}sq")
                nc.vector.tensor_mul(sq[:], qs[:], qs[:])
                rstd = rope.tile([P, NHL], F32, name=f"{t}rstd{ibg}", tag=f"{t}rstd")
                nc.vector.reduce_sum(
                    rstd[:],
                    sq[:].rearrange("p (h d) -> p h d", h=NHL),
                    axis=mybir.AxisListType.X,
                )
                nc.scalar.activation(
                    rstd[:], rstd[:],
                    mybir.ActivationFunctionType.Sqrt,
                    bias=eps_t[:], scale=1.0 / HD,
                )
                nc.vector.reciprocal(rstd[:], rstd[:])

                q3 = qs[:].rearrange("p (h d) -> p h d", h=NHL)
                qr = rope.tile([P, GW], F16, name=f"{t}r{ibg}", tag=f"{t}r")
                qr3 = qr[:].rearrange("p (h d) -> p h d", h=NHL)
                tmp = rope.tile([P, NHL, HD // 2], F16, name=f"{t}tmp{ibg}", tag=f"{t}tmp")
                cosB = cos_sb[:, ibg:ibg + 1, :].broadcast_to((P, NHL, HD // 2))
                sinB = sin_sb[:, ibg:ibg + 1, :].broadcast_to((P, NHL, HD // 2))
                h1 = q3[:, :, 0:HD // 2]
                h2 = q3[:, :, HD // 2:HD]
                # r1 = q1*cos + q2*sin ; r2 = q2*cos - q1*sin
                nc.vector.tensor_mul(qr3[:, :, 0:HD // 2], h1, cosB)
                nc.vector.tensor_mul(tmp[:], h2, sinB)
                nc.vector.tensor_add(qr3[:, :, 0:HD // 2], qr3[:, :, 0:HD // 2], tmp[:])
                nc.vector.tensor_mul(qr3[:, :, HD // 2:HD], h2, cosB)
                nc.vector.tensor_mul(tmp[:], h1, sinB)
                nc.vector.tensor_sub(
                    qr3[:, :, HD // 2:HD], qr3[:, :, HD // 2:HD], tmp[:]
                )
                for h in range(NHL):
                    nc.vector.tensor_scalar_mul(
                        qr[:, h * HD:(h + 1) * HD],
                        qr[:, h * HD:(h + 1) * HD],
                        rstd[:, h:h + 1],
                    )
                return qr

            # ---------------- Phase A: Q, K, V in one xt pass ----------------
            with ExitStack() as pha:
                wpool = pha.enter_context(tc.tile_pool(name="wpool", bufs=1))
                xt_pool = pha.enter_context(tc.tile_pool(name="xt_pool", bufs=2))
                proj_ps = pha.enter_context(tc.tile_pool(name="proj_ps", bufs=3, space="PSUM"))
                tp_ps = pha.enter_context(tc.tile_pool(name="tp_ps", bufs=2, space="PSUM"))
                rope = pha.enter_context(tc.tile_pool(name="rope", bufs=2))

                wq_sb = wpool.tile([P, CT, GW], F16, name="wq_sb")
                wk_sb = wpool.tile([P, CT, GW], F16, name="wk_sb")
                wv_sb = wpool.tile([P, CT, GW], F16, name="wv_sb")

                for ica in range(8):
                    # three parallel DMA rings at startup: wq/wk on sync,
                    # xt on scalar, wv/wo/cos/sin on gpsimd
                    xt_ch = xt_pool.tile([P, CT, 256], F16, name=f"xt_ch{ica}", tag="xt")
                    for ct in range(CT):
                        if ica == 0:
                            nc.sync.dma_start(out=wq_sb[:, ct, :], in_=wq_in[ct * P:(ct + 1) * P, :])
                            nc.gpsimd.dma_start(out=wv_sb[:, ct, :], in_=wv_in[ct * P:(ct + 1) * P, :])
                        nc.scalar.dma_start(
                            out=xt_ch[:, ct, :],
                            in_=xt_in[ct * P:(ct + 1) * P, ica * 256:(ica + 1) * 256],
                        )
                    if ica == 0:
                        for ct in range(CT):
                            nc.sync.dma_start(out=wk_sb[:, ct, :], in_=wk_in[ct * P:(ct + 1) * P, :])
                        nc.gpsimd.dma_start(
                            out=cos_sb[:], in_=cos_in.ap().rearrange("(a p) f -> p a f", p=P)
                        )
                        nc.gpsimd.dma_start(
                            out=sin_sb[:], in_=sin_in.ap().rearrange("(a p) f -> p a f", p=P)
                        )
                        for ct in range(CT):
                            nc.gpsimd.dma_start(out=wo_sb[:, ct, :], in_=wo_in[ct * P:(ct + 1) * P, :])

                    for ib in range(2):
                        ibg = ica * 2 + ib

                        ps_q = proj_ps.tile([P, GW], F32, name=f"psq{ibg}", tag="proj")
                        for ct in range(CT):
                            nc.tensor.matmul(
                                ps_q[:], xt_ch[:, ct, ib * P:(ib + 1) * P], wq_sb[:, ct, :],
                                start=(ct == 0), stop=(ct == CT - 1),
                            )
                        ps_k = proj_ps.tile([P, GW], F32, name=f"psk{ibg}", tag="proj")
                        for ct in range(CT):
                            nc.tensor.matmul(
                                ps_k[:], xt_ch[:, ct, ib * P:(ib + 1) * P], wk_sb[:, ct, :],
                                start=(ct == 0), stop=(ct == CT - 1),
                            )
                        # q's vector chain runs while V streams on the PE
                        qr = norm_rope(rope, ps_q, ibg, "q")
                        ps_v = proj_ps.tile([P, GW], F32, name=f"psv{ibg}", tag="proj")
                        for ct in range(CT):
                            nc.tensor.matmul(
                                ps_v[:], xt_ch[:, ct, ib * P:(ib + 1) * P], wv_sb[:, ct, :],
                                start=(ct == 0), stop=(ct == CT - 1),
                            )
                        kr = norm_rope(rope, ps_k, ibg, "k")
                        nc.vector.tensor_copy(v_sb[:, ibg, :], ps_v[:])
                        for h in range(NHL):
                            tp = tp_ps.tile([P, P], F16, name=f"tpq{ibg}_{h}", tag="tp")
                            nc.tensor.transpose(tp[:], qr[:, h * HD:(h + 1) * HD], ident[:])
                            nc.scalar.copy(qt_sb[:, h, ibg * P:(ibg + 1) * P], tp[:])
                        for h in range(NHL):
                            tp = tp_ps.tile([P, P], F16, name=f"tpk{ibg}_{h}", tag="tp")
                            nc.tensor.transpose(tp[:], kr[:, h * HD:(h + 1) * HD], ident[:])
                            nc.scalar.copy(kt_sb[:, h, ibg * P:(ibg + 1) * P], tp[:])

            # ---------------- Phase B: attention ----------------
            with ExitStack() as phb:
                et_pool = phb.enter_context(tc.tile_pool(name="et_pool", bufs=5))
                bsmall = phb.enter_context(tc.tile_pool(name="bsmall", bufs=2))
                # all 4 heads' normalized outputs stay live until the chunk's
                # deferred DMA batch fires
                ytsb_pool = phb.enter_context(tc.tile_pool(name="ytsb_pool", bufs=5))
                # ag tiles prefetched during B, consumed by phase D
                ag_pool = phb.enter_context(tc.tile_pool(name="ag_pool", bufs=4))
                ag_tiles = {}
                s_ps = phb.enter_context(tc.tile_pool(name="s_ps", bufs=4, space="PSUM"))
                yt_psp = phb.enter_context(tc.tile_pool(name="yt_psp", bufs=2, space="PSUM"))
                den_psp = phb.enter_context(tc.tile_pool(name="den_psp", bufs=2, space="PSUM"))

                def emit_d_load(icc):
                    # 4 mt-blocks per descriptor: few sync-engine triggers while
                    # still letting o_proj start before the full tile lands
                    ag_ch = ag_pool.tile([P, CT, 512], F16, name=f"ag{icc}", tag="ag")
                    for k in range(4):
                        # chunk 3's load is the exposed tail: split it across
                        # the sync and gpsimd rings for double burst bandwidth
                        # (gpsimd's queue is empty after the last AllGather)
                        eng = nc.gpsimd if (icc == 3 and k % 2 == 1) else nc.sync
                        eng.dma_start(
                            out=ag_ch[:, 4 * k:4 * (k + 1), :],
                            in_=ag_ics[icc][k * 512:(k + 1) * 512, :]
                                .rearrange("(t p) f -> p t f", p=P),
                        )
                    ag_tiles[icc] = ag_ch

                for ic in range(ICH):
                    njb = 4 * ic + 4
                    pending_drain = None
                    pending_drain_h = -1
                    for h in range(NHL):
                        yt_ps = yt_psp.tile([P, 512], F32, name=f"yt{ic}_{h}", tag="yt")
                        den_ps = den_psp.tile([P, 512], F32, name=f"den{ic}_{h}", tag="den")
                        sps = {}

                        def emit_score(jb, h=h, ic=ic, sps=sps):
                            off = max(0, P * (jb - 4 * ic))
                            sp = s_ps.tile([P, 512], F32, name=f"s{ic}_{h}_{jb}", tag="s")
                            nc.tensor.matmul(
                                sp[:, off:512],
                                kt_sb[:, h, jb * P:(jb + 1) * P],
                                qt_sb[:, h, ic * 512 + off:(ic + 1) * 512],
                                start=True, stop=True,
                            )
                            sps[jb] = sp

                        def emit_finish(jb, h=h, ic=ic, njb=njb, sps=sps,
                                        yt_ps=yt_ps, den_ps=den_ps):
                            off = max(0, P * (jb - 4 * ic))
                            sp = sps.pop(jb)
                            et = et_pool.tile([P, 512], F16, name=f"et{ic}_{h}_{jb}", tag="et")
                            nc.scalar.activation(
                                et[:, off:512], sp[:, off:512],
                                mybir.ActivationFunctionType.Exp,
                                bias=neg1_t[:], scale=SCALE,
                            )
                            if jb >= 4 * ic:
                                # diagonal sub-block: zero the upper triangle
                                nc.vector.tensor_mul(
                                    et[:, off:off + P], et[:, off:off + P], tri_m[:]
                                )
                            nc.tensor.matmul(
                                yt_ps[:, off:512],
                                v_sb[:, jb, h * HD:(h + 1) * HD],
                                et[:, off:512],
                                start=(jb == 0), stop=(jb == njb - 1),
                            )
                            nc.tensor.matmul(
                                den_ps[:, off:512],
                                ones16[:],
                                et[:, off:512],
                                start=(jb == 0), stop=(jb == njb - 1),
                            )

                        # three scores of lookahead before the previous head's
                        # drain and before each finish
                        LA = min(3, njb)
                        for jb in range(LA):
                            emit_score(jb)
                        if pending_drain is not None:
                            pending_drain()
                            pending_drain = None
                        for jb in range(LA, njb):
                            emit_score(jb)
                            emit_finish(jb - LA)
                        for jb in range(njb - LA, njb):
                            emit_finish(jb)

                        def make_drain(h=h, ic=ic, yt_ps=yt_ps, den_ps=den_ps):
                            def drain():
                                # den is already replicated across partitions:
                                # one fast approx reciprocal + one multiply
                                rden = bsmall.tile([P, 512], F32, name=f"rd{ic}_{h}", tag="rden")
                                nc.vector.reciprocal_approx_fast(out=rden[:], in_=den_ps[:])
                                yt_sb = ytsb_pool.tile([P, 512], F16, name=f"yts{ic}_{h}", tag="yts")
                                nc.vector.tensor_mul(yt_sb[:], yt_ps[:], rden[:])
                                nc.gpsimd.dma_start(
                                    out=yt_ics[ic][h * P:(h + 1) * P, :], in_=yt_sb[:]
                                )
                            return drain
                        pending_drain = make_drain()
                        pending_drain_h = h

                    pending_drain()
                    pending_drain = None
                    # per-chunk AllGather within the batch group of 4 cores
                    nc.gpsimd.collective_compute(
                        "AllGather",
                        mybir.AluOpType.bypass,
                        replica_groups=[[0, 1, 2, 3], [4, 5, 6, 7]],
                        ins=[yt_ics[ic][:].opt()],
                        outs=[ag_ics[ic][:].opt()],
                    )
                    # prefetch the gathered tile for o_proj immediately: the
                    # triggers wait on the AllGather semaphore and transfer
                    # before later AllGathers contend for the interconnect
                    emit_d_load(ic)

                # ---------------- Phase D: o_proj, after all attention ----------------
                # no new pools here: a pool-alloc boundary after the collectives
                # would wait on them and stall the PE.  o_proj accumulators
                # alternate between the attention pools' yt/den PSUM tags.
                for icc in range(ICH):
                    ag_ch = ag_tiles.pop(icc)
                    for oc in range(4):
                        psp = yt_psp if oc % 2 == 0 else den_psp
                        y_ps = psp.tile([P, 512], F32, name=f"yp{icc}_{oc}",
                                        tag="yt" if oc % 2 == 0 else "den")
                        for j, mt in enumerate(range(CT)):
                            nc.tensor.matmul(
                                y_ps[:],
                                wo_sb[:, mt, oc * P:(oc + 1) * P],
                                ag_ch[:, mt, :],
                                start=(j == 0), stop=(j == CT - 1),
                            )
                        y_sb = ytsb_pool.tile([P, 512], F16, name=f"ysb{icc}_{oc}", tag="ysb")
                        nc.vector.tensor_copy(y_sb[:], y_ps[:])
                        nc.scalar.dma_start(
                            out=yt_out[oc * P:(oc + 1) * P, icc * 512:(icc + 1) * 512],
                            in_=y_sb[:],
                        )

    nc.compile()
    _program_cache["nc"] = nc
    return nc


def _rope_tables():
    inv_freq = 1.0 / (10000.0 ** (np.arange(0, HD, 2, dtype=np.float32) / HD))
    pos = np.arange(S, dtype=np.float32)
    freqs = np.outer(pos, inv_freq).astype(np.float32)
    return np.cos(freqs).astype(np.float16), np.sin(freqs).astype(np.float16)


def make_in_maps(x, Wq, Wk, Wv, Wo):
    x = np.asarray(x, dtype=np.float32)
    cos, sin = _rope_tables()
    wqT = np.ascontiguousarray(np.asarray(Wq, dtype=np.float32).T.astype(np.float16))
    wkT = np.ascontiguousarray(np.asarray(Wk, dtype=np.float32).T.astype(np.float16))
    wvT = np.ascontiguousarray(np.asarray(Wv, dtype=np.float32).T.astype(np.float16))
    woT = np.ascontiguousarray(np.asarray(Wo, dtype=np.float32).T.astype(np.float16))
    xts = [np.ascontiguousarray(x[b].T.astype(np.float16)) for b in range(2)]
    in_maps = []
    for c in range(NCORES):
        b, g = c // 4, c % 4
        sl = slice(g * GW, (g + 1) * GW)
        in_maps.append({
            "xt": xts[b],
            "wq": np.ascontiguousarray(wqT[:, sl]),
            "wk": np.ascontiguousarray(wkT[:, sl]),
            "wv": np.ascontiguousarray(wvT[:, sl]),
            "wo": np.ascontiguousarray(woT[:, sl]),
            "cos": cos,
            "sin": sin,
        })
    return in_maps


def assemble_output(results):
    y = np.empty((2, S, D), dtype=np.float32)
    for c in range(NCORES):
        b, g = c // 4, c % 4
        y[b][:, g * GW:(g + 1) * GW] = results[c]["yt_out"].T
    return y


def kernel(x, Wq, Wk, Wv, Wo):
    nc = build_program()
    in_maps = make_in_maps(x, Wq, Wk, Wv, Wo)
    res = run_bass_kernel_spmd(nc, in_maps, core_ids=list(range(NCORES)))
    return assemble_output(res.results)
